# revision 2
# baseline (speedup 1.0000x reference)
"""
Muskingum-Cunge river routing over a 14-level binary confluence tree,
T=2048 timesteps x 4 substeps, on 8 Trainium2 NeuronCores. v4.

v4 vs v2: the warm-start trajectory admits only TWO distinct coefficient
sets per timestep: substep 0 (Qref from 2*I_prev + I_cur) and substeps 1-3
(Qref = I_cur).  The four substep affine maps compose in closed form into
one per-timestep map:

    A = a1^3 * a0
    B = a1^3 * b0 + b1 * (1 + a1 + a1^2)

so the coefficient pipeline runs on a [128, 2L] concat grid (slot-0 |
slot-123) -- 2.5 G-pass equivalents of ACT work instead of 5 -- and the
hardware scan runs at T resolution instead of 4T.  Offline maxrel 1.26e-3
(vs v2's 2.6e-3 and the 2e-2 gate).

Levels 0..4 run as 128-row chunks with the T axis split into 2 half-slabs
for software pipelining; levels 5..13 time-split (reach r's T axis cut
into F segments on partitions r*F+s) with per-segment prefix (Apre,Bpre)
and a 128-wide cross-partition stitch (PE transpose + 1-row scan).

Sharding: each core owns one complete subtree (contiguous 1/8 slice of
every level 0..10); one AllGather of the 8 level-10 roots; levels 11-13
computed redundantly on every core.
"""

import sys
import numpy as np

for _p in ("/opt/trn_rl_repo", "/root/.axon_site/_ro/trn_rl_repo"):
    if _p not in sys.path:
        sys.path.append(_p)

import concourse.bass as bass
import concourse.mybir as mybir
from concourse import bacc, tile
from concourse.bass_types import AP
from concourse.bass_utils import run_bass_kernel_spmd
from concourse.masks import make_identity

F32 = mybir.dt.float32
BF16 = mybir.dt.bfloat16
AF = mybir.ActivationFunctionType
ALU = mybir.AluOpType

N_LEVELS = 14
LS = [8192 >> l for l in range(N_LEVELS)]
LO = [0]
for _s in LS:
    LO.append(LO[-1] + _s)
T = 2048
DT_SUB = 86400.0 / 4
EPS = 1e-6
LNDT = float(np.log(np.float32(DT_SUB)))
LN53 = float(np.log(np.float32(5.0 / 3.0)))
NCORES = 8
SLAB = 1024          # half of the T grid
NSLAB = T // SLAB    # 2
PAD = 8
NPH = 7              # phases per slab

SZC = [LS[l] // NCORES for l in range(11)]
N_STD = 5
SPLIT_ROWS = {l: (SZC[l] if l < 11 else LS[l]) for l in range(N_STD, 14)}
SPLIT_F = {l: 128 // SPLIT_ROWS[l] for l in range(N_STD, 14)}
GROUP = 2  # software-pipeline depth


def _ap3(dram_ap, off, dims):
    return AP(dram_ap.tensor, dram_ap.offset + off, dims)


def _lean_consts(nc, tiny, prm_view, rows):
    """[rows,7] params -> (negp, r, lgh, lgg) [128,1] f32 APs (lean form)."""
    prm = tiny.tile([128, 8], F32, tag="prm", name="prm")
    if rows < 128:
        nc.vector.memset(prm[rows:, :], 1.0)
    nc.sync.dma_start(prm[:rows, 0:7], prm_view)
    rows = 128
    lgn = prm[:rows, 0:1]
    dx, S, wc = prm[:rows, 1:2], prm[:rows, 2:3], prm[:rows, 3:4]
    we, dc, de = prm[:rows, 4:5], prm[:rows, 5:6], prm[:rows, 6:7]

    def tt(name):
        return tiny.tile([128, 1], F32, tag=name, name=name)

    lgS, lgdc, lgdx, lgwc = tt("c1"), tt("c2"), tt("c3"), tt("c4")
    nc.scalar.activation(lgS[:rows, :], S, AF.Ln)
    nc.scalar.activation(lgdc[:rows, :], dc, AF.Ln)
    nc.scalar.activation(lgdx[:rows, :], dx, AF.Ln)
    nc.scalar.activation(lgwc[:rows, :], wc, AF.Ln)
    p, negp, r = tt("c5"), tt("c6"), tt("c7")
    nc.vector.tensor_scalar_mul(p[:rows, :], de, 2.0 / 3.0)
    nc.vector.tensor_scalar_mul(negp[:rows, :], p[:rows, :], -1.0)
    nc.vector.scalar_tensor_tensor(r[:rows, :], p[:rows, :], -2.0, we,
                                   ALU.mult, ALU.subtract)
    nc.vector.tensor_scalar_add(r[:rows, :], r[:rows, :], 1.0)
    lgB = tt("c8")
    nc.vector.tensor_scalar_mul(lgB[:rows, :], lgdc[:rows, :], 2.0 / 3.0)
    nc.vector.scalar_tensor_tensor(lgB[:rows, :], lgS[:rows, :], 0.5,
                                   lgB[:rows, :], ALU.mult, ALU.add)
    nc.vector.tensor_tensor(lgB[:rows, :], lgB[:rows, :], lgn, ALU.subtract)
    nc.vector.tensor_scalar_add(lgB[:rows, :], lgB[:rows, :], LN53)
    lgh = tt("c9")
    nc.vector.tensor_tensor(lgh[:rows, :], lgdx[:rows, :], lgB[:rows, :],
                            ALU.subtract)
    lgg = tt("c11")
    nc.vector.scalar_tensor_tensor(lgg[:rows, :], lgB[:rows, :], -2.0,
                                   lgwc[:rows, :], ALU.mult, ALU.subtract)
    nc.vector.tensor_tensor(lgg[:rows, :], lgg[:rows, :], lgS[:rows, :],
                            ALU.subtract)
    # fold 1/DT into the K/N island so the division island's log bias is 1
    # (R = 1/(Ds' + 1) with Ds' = Ds/DT) and the substep-0 correction needs
    # the raw (I_old - I_new) instead of (I_old - I_new)/DT.
    nc.vector.tensor_scalar_add(lgh[:rows, :], lgh[:rows, :], -LNDT)
    nc.vector.tensor_scalar_add(lgg[:rows, :], lgg[:rows, :], -LNDT)
    return (negp[:rows, :], r[:rows, :], lgh[:rows, :], lgg[:rows, :])


def _coeff_phases(nc, temps, consts, iv, ish, dinp_v, L, scan_fn):
    """One slab ([128, L] on the T grid) of v4 coefficient math.

    Concat layout: the coefficient island runs on [128, 2L] tiles whose
    first L columns are substep-0 quantities and last L columns are the
    substep-123 quantities.  iv/ish: inflow and shifted inflow [128, L]
    f32.  dinp_v: (I_old-I_new)/DT [128, L] bf16.  scan_fn(A, B) emits the
    scan(s) consuming the composed per-timestep coefficients."""
    negp, r_ap, lgh_ap, lgg_ap = consts
    L2 = 2 * L
    st = {}

    def tl(tag, n, dt=F32, bufs=None):
        t = temps.tile([128, n], dt, tag=tag, name=tag, bufs=bufs)
        return t[:, 0:n]

    def phA():  # DVE: slot-0 sarg cc = 2*ish + iv; ACT: slot-123 log
        st["cc"] = cc = tl("cc", L, bufs=GROUP)
        nc.vector.scalar_tensor_tensor(cc, ish, 2.0, iv, ALU.mult, ALU.add)
        st["lg"] = lg = tl("lg", L2, bufs=GROUP)
        nc.scalar.activation(lg[:, L:L2], iv, AF.Ln, scale=1.0, bias=EPS)

    def phB():  # ACT: slot-0 log into the concat lg tile
        lg = st["lg"]
        nc.scalar.activation(lg[:, 0:L], st["cc"], AF.Ln, scale=1.0 / 3.0,
                             bias=EPS)

    def phC():  # ACT: K/N island (bf16)
        st["Kb"] = Kb = tl("Kb", L2, BF16, bufs=GROUP)
        nc.scalar.activation(Kb, st["lg"], AF.Exp, scale=negp, bias=lgh_ap)
        st["Nb"] = Nb = tl("Nb", L2, BF16, bufs=GROUP)
        nc.scalar.activation(Nb, st["lg"], AF.Exp, scale=r_ap, bias=lgg_ap)

    def phD():  # DVE: M = min(Nb, Kb) (bf16 2x); Pool: Ds = Kb + M
        st["M"] = M = tl("M", L2, BF16, bufs=GROUP)
        nc.vector.tensor_tensor(M, st["Nb"], st["Kb"], ALU.min)
        st["Ds"] = Ds = tl("Ds", L2, BF16, bufs=GROUP)
        nc.gpsimd.tensor_tensor(Ds, st["Kb"], M, ALU.add)

    def phE():  # ACT: division island  R = 1 / (Ds' + 1)
        lgD = tl("lg", L2, bufs=GROUP)   # reuse lg rotation
        nc.scalar.activation(lgD, st["Ds"], AF.Ln, bias=1.0)
        st["R"] = R = tl("R", L2, bufs=GROUP)
        nc.scalar.activation(R, lgD, AF.Exp, scale=-1.0)

    def phF():  # ACT: a = 1-2R; Pool: KN chain; DVE: b0, b1
        R = st["R"]
        R0, R1 = R[:, 0:L], R[:, L:L2]
        st["a"] = a = tl("a", L2, bufs=GROUP)
        nc.scalar.activation(a, R, AF.Identity, scale=-2.0, bias=1.0)
        st["KN"] = KN = tl("KN", L, BF16)
        nc.gpsimd.tensor_tensor(KN, st["Kb"][:, 0:L], st["M"][:, 0:L],
                                ALU.subtract)
        KN2 = tl("u", L)
        nc.gpsimd.tensor_tensor(KN2, KN, R0, ALU.mult)
        KN3 = tl("kn3", L)
        nc.gpsimd.tensor_tensor(KN3, KN2, dinp_v, ALU.mult)
        u = tl("u", L)
        nc.vector.tensor_tensor(u, ish, iv, ALU.add)
        st["b0"] = b0 = tl("b0", L, bufs=GROUP)
        nc.vector.tensor_tensor(b0, u, R0, ALU.mult)
        nc.gpsimd.tensor_tensor(b0, b0, KN3, ALU.add)
        st["b1"] = b1 = tl("b1", L, bufs=GROUP)
        nc.vector.scalar_tensor_tensor(b1, iv, 2.0, R1, ALU.mult, ALU.mult)

    def phG():  # DVE: composition + scan; Pool: the two adds
        a = st["a"]
        a0, a1 = a[:, 0:L], a[:, L:2 * L]
        asq = tl("cc", L)     # reuse cc rotation
        nc.vector.tensor_tensor(asq, a1, a1, ALU.mult)
        t3 = tl("lg", L)      # reuse lg rotation
        nc.vector.tensor_tensor(t3, asq, a1, ALU.mult)
        A = tl("R", L)        # reuse R rotation
        nc.vector.tensor_tensor(A, t3, st["a"][:, 0:L], ALU.mult)
        B = tl("B", L, bufs=GROUP)
        nc.vector.tensor_tensor(B, t3, st["b0"], ALU.mult)
        pe = tl("u", L)       # reuse u rotation
        nc.vector.scalar_tensor_tensor(pe, a1, 1.0, asq, ALU.add, ALU.add)
        B2 = tl("lg", L)      # reuse lg rotation again
        nc.gpsimd.tensor_tensor(B2, st["b1"], pe, ALU.mult)
        nc.gpsimd.tensor_tensor(B, B, B2, ALU.add)
        scan_fn(A, B)

    return [phA, phB, phC, phD, phE, phF, phG]


def _run_pipelined(phase_lists):
    for i in range(0, len(phase_lists), GROUP):
        grp = phase_lists[i:i + GROUP]
        for p in range(NPH):
            for ph in grp:
                ph[p]()


def _std_chunk(nc, pools, consts, lat_dram, prev_q_dram, out_q_dram,
               out_padded, c, rows=128):
    """Standard chunk (levels 0..4): [128, T] grid, NSLAB half-slabs."""
    pers, temps, tiny = pools

    ibuf = pers.tile([128, PAD + T], F32, tag="ibuf", name="ibuf", bufs=2)
    nc.vector.memset(ibuf[:, 0:PAD], 0.0)
    if rows < 128:  # keep junk partitions finite
        nc.vector.memset(ibuf[rows:, :], 1.0)
    infl = ibuf[:, PAD:PAD + T]
    infl_sh = ibuf[:, PAD - 1:PAD - 1 + T]
    nc.sync.dma_start(infl[:rows, :], lat_dram[c * rows:(c + 1) * rows, :])
    if prev_q_dram is not None:
        qe = temps.tile([128, T], F32, tag="qe", name="qe", bufs=1)
        qo = temps.tile([128, T], F32, tag="qo", name="qo", bufs=1)
        r0 = 2 * c * rows
        nc.sync.dma_start(qe[:rows, :], prev_q_dram[r0:r0 + 2 * rows:2, :])
        nc.sync.dma_start(qo[:rows, :], prev_q_dram[r0 + 1:r0 + 2 * rows:2, :])
        nc.gpsimd.tensor_tensor(infl[:rows, :], infl[:rows, :], qe[:rows, :],
                                ALU.add)
        nc.gpsimd.tensor_tensor(infl[:rows, :], infl[:rows, :], qo[:rows, :],
                                ALU.add)

    dinp = pers.tile([128, T], BF16, tag="dinp", name="dinp", bufs=2)
    nc.gpsimd.tensor_tensor(dinp[:, :], infl_sh, infl, ALU.subtract)

    z = pers.tile([128, T], F32, tag="z", name="z", bufs=1)

    phase_lists = []
    for sl in range(NSLAB):
        g0 = sl * SLAB

        def scan_fn(A, B, sl=sl, g0=g0):
            init = 0.0 if sl == 0 else z[:, g0 - 1:g0]
            nc.vector.tensor_tensor_scan(z[:, g0:g0 + SLAB], A, B, init,
                                         ALU.mult, ALU.add)

        phase_lists.append(_coeff_phases(
            nc, temps, consts, infl[:, g0:g0 + SLAB],
            infl_sh[:, g0:g0 + SLAB], dinp[:, g0:g0 + SLAB], SLAB, scan_fn))

    def finalize():
        qout = temps.tile([128, T], F32, tag="qe", name="qout", bufs=1)
        nc.scalar.activation(qout[:, :], z[:, :], AF.Relu)
        col0 = 1 if out_padded else 0
        nc.sync.dma_start(
            out_q_dram[c * rows:(c + 1) * rows, col0:col0 + T], qout[:rows, :])

    return phase_lists, finalize


def _split_level(nc, pools, psum, consts, ident, lat_dram, prev_q_ap_fn,
                 out_write_fn, l):
    """Time-split level solve: R reaches x F segments on 128 partitions."""
    pers, temps, tiny = pools
    R = SPLIT_ROWS[l]
    F = SPLIT_F[l]
    Tseg = T // F

    ibuf = pers.tile([128, PAD + T], F32, tag="ibuf", name="ibuf_s", bufs=2)
    iv_full = ibuf[:, 0:Tseg + 1]
    nc.sync.dma_start(iv_full, lat_dram[:, :])
    if prev_q_ap_fn is not None:
        qe = temps.tile([128, T], F32, tag="qe", name="qe_s", bufs=1)
        qo = temps.tile([128, T], F32, tag="qo", name="qo_s", bufs=1)
        nc.sync.dma_start(qe[:, 0:Tseg + 1], prev_q_ap_fn(0))
        nc.sync.dma_start(qo[:, 0:Tseg + 1], prev_q_ap_fn(1))
        nc.gpsimd.tensor_tensor(iv_full, iv_full, qe[:, 0:Tseg + 1], ALU.add)
        nc.gpsimd.tensor_tensor(iv_full, iv_full, qo[:, 0:Tseg + 1], ALU.add)
    infl = ibuf[:, 1:Tseg + 1]
    infl_sh = ibuf[:, 0:Tseg]

    dinp = pers.tile([128, T], BF16, tag="dinp", name="dinp_s", bufs=2)
    dv = dinp[:, 0:Tseg]
    nc.gpsimd.tensor_tensor(dv, infl_sh, infl, ALU.subtract)

    Apre = pers.tile([128, SLAB], F32, tag="Apre", name="Apre")
    Bpre = pers.tile([128, SLAB], F32, tag="Bpre", name="Bpre")

    bnd = tiny.tile([128, 2], F32, tag="bnd", name="bnd")
    bndTA = psum.tile([1, 128], F32, tag="bndTA", name="bndTA")
    bndTB = psum.tile([1, 128], F32, tag="bndTB", name="bndTB")
    scanA = tiny.tile([1, 128], F32, tag="scanA", name="scanA")
    scanB = tiny.tile([1, 128], F32, tag="scanB", name="scanB")
    zrow = tiny.tile([1, 128], F32, tag="zrow", name="zrow")
    zinT = psum.tile([128, 1], F32, tag="zinT", name="zinT")
    zin = tiny.tile([128, 1], F32, tag="zin", name="zin")

    def scan_fn(A, B):
        nc.vector.tensor_tensor_scan(Apre[:, 0:Tseg], A, A, 1.0,
                                     ALU.mult, ALU.bypass)
        nc.vector.tensor_tensor_scan(Bpre[:, 0:Tseg], A, B, 0.0,
                                     ALU.mult, ALU.add)

    # split the segment axis into two half-slabs when long enough so the
    # coefficient phases of one half overlap the other's scans
    if Tseg >= 256:
        h = Tseg // 2
        phase_lists = []
        for sl, (c0, cn) in enumerate(((0, h), (h, Tseg - h))):
            def scan_fn_h(A, B, sl=sl, c0=c0, cn=cn):
                initA = 1.0 if sl == 0 else Apre[:, c0 - 1:c0]
                nc.vector.tensor_tensor_scan(Apre[:, c0:c0 + cn], A, A,
                                             initA, ALU.mult, ALU.bypass)
                initB = 0.0 if sl == 0 else Bpre[:, c0 - 1:c0]
                nc.vector.tensor_tensor_scan(Bpre[:, c0:c0 + cn], A, B,
                                             initB, ALU.mult, ALU.add)

            phase_lists.append(_coeff_phases(
                nc, temps, consts, infl[:, c0:c0 + cn],
                infl_sh[:, c0:c0 + cn], dv[:, c0:c0 + cn], cn, scan_fn_h))
    else:
        phase_lists = [_coeff_phases(nc, temps, consts, infl, infl_sh, dv,
                                     Tseg, scan_fn)]
    _run_pipelined(phase_lists)

    # stitch segment boundaries: zin[p] = z entering segment p
    nc.vector.tensor_copy(bnd[:, 0:1], Apre[:, Tseg - 1:Tseg])
    nc.vector.tensor_copy(bnd[:, 1:2], Bpre[:, Tseg - 1:Tseg])
    nc.tensor.transpose(bndTA[:, :], bnd[:, 0:1], ident[:, :])
    nc.tensor.transpose(bndTB[:, :], bnd[:, 1:2], ident[:, :])
    nc.vector.memset(scanA[:, 0:1], 0.0)
    nc.vector.memset(scanB[:, 0:1], 0.0)
    nc.vector.tensor_copy(scanA[:, 1:128], bndTA[0:1, 0:127])
    nc.vector.tensor_copy(scanB[:, 1:128], bndTB[0:1, 0:127])
    if R > 1:
        nc.vector.memset(scanA[:, 0::F], 0.0)
        nc.vector.memset(scanB[:, 0::F], 0.0)
    nc.vector.tensor_tensor_scan(zrow[:, :], scanA[:, :],
                                 scanB[:, :], 0.0, ALU.mult, ALU.add)
    nc.tensor.transpose(zinT[:, :], zrow[:, :], ident[0:1, 0:1])
    nc.vector.tensor_copy(zin[:, :], zinT[:, :])

    qex = temps.tile([128, T], F32, tag="qo", name="qex", bufs=1)
    qv = qex[:, 0:Tseg]
    nc.vector.scalar_tensor_tensor(qv, Apre[:, 0:Tseg], zin[:, 0:1],
                                   Bpre[:, 0:Tseg], ALU.mult, ALU.add)
    nc.vector.tensor_scalar(qv, qv, 0.0, None, ALU.max)
    out_write_fn(qv)


def _patch_act_tables():
    """Restrict the activation-table list to the one set containing every
    function this kernel uses (Ln, Exp, Relu), avoiding table thrash."""
    import concourse.hw_specs as hw_specs
    import concourse.bacc as bacc_mod
    orig = hw_specs.get_activation_tables.__wrapped__

    def patched(module_arch):
        tabs = orig(module_arch)
        if "natural_log_exp_and_others" not in tabs:
            return tabs
        return {k: (v if k == "natural_log_exp_and_others" else set())
                for k, v in tabs.items()}

    import functools
    wrapped = functools.cache(patched)
    hw_specs.get_activation_tables = wrapped
    bacc_mod.get_activation_tables = wrapped


def _build_program(timeline=False, levels=None):
    emit = set(range(14)) if levels is None else set(levels)
    _patch_act_tables()
    nc = bacc.Bacc("TRN2", target_bir_lowering=False, debug=False,
                   num_devices=1 if timeline else NCORES)
    for name, val in (("c-eps", EPS), ("c-dt", DT_SUB), ("c-lndt", LNDT)):
        cb = nc.alloc_sbuf_tensor(name, [128, 1], F32)
        nc.gpsimd.memset(cb.ap(), val)
        nc.const_aps.aps[(F32, val)] = cb.ap()
    nc.all_engine_barrier()

    lat_d, prm_d = {}, {}
    for l in range(N_STD):
        lat_d[l] = nc.declare_dram_parameter(f"lat{l}", [SZC[l], T], F32,
                                             isOutput=False)
        prm_d[l] = nc.declare_dram_parameter(f"prm{l}", [SZC[l], 7], F32,
                                             isOutput=False)
    for l in range(N_STD, 14):
        Tseg = T // SPLIT_F[l]
        lat_d[l] = nc.declare_dram_parameter(f"lat{l}", [128, Tseg + 1], F32,
                                             isOutput=False)
        prm_d[l] = nc.declare_dram_parameter(f"prm{l}", [128, 7], F32,
                                             isOutput=False)
    outlet = nc.declare_dram_parameter("outlet", [1, T], F32, isOutput=True)

    with tile.TileContext(nc) as tc:
        import contextlib
        with contextlib.ExitStack() as ctx:
            pers = ctx.enter_context(tc.tile_pool(name="pers", bufs=1))
            temps = ctx.enter_context(tc.tile_pool(name="temps", bufs=2))
            tiny = ctx.enter_context(tc.tile_pool(name="tiny", bufs=2))
            psum = ctx.enter_context(tc.tile_pool(name="psum", bufs=2,
                                                  space="PSUM"))
            dram = ctx.enter_context(tc.tile_pool(name="dram", bufs=1,
                                                  space="DRAM"))
            pools = (pers, temps, tiny)

            ident = pers.tile([128, 128], F32, tag="ident", name="ident")
            make_identity(nc, ident[:, :])
            zcol = pers.tile([128, 1], F32, tag="zcol", name="zcol")
            nc.vector.memset(zcol[:, :], 0.0)

            # DRAM q buffers: q0..q3 unpadded; q4..q9, q11, q12, gathp padded
            q = {}
            for l in range(N_STD - 1):
                q[l] = dram.tile([SZC[l], T], F32, tag=f"q{l}", name=f"q{l}")
            for l in range(N_STD - 1, 10):
                q[l] = dram.tile([SZC[l], T + 1], F32, tag=f"q{l}",
                                 name=f"q{l}")
                nc.sync.dma_start(q[l][:, 0:1], zcol[0:SZC[l], :])
            q[10] = dram.tile([1, T], F32, tag="q10", name="q10")
            gath = dram.tile([NCORES, T], F32, tag="gath", name="gath")
            gathp = dram.tile([NCORES, T + 1], F32, tag="gathp", name="gathp")
            nc.sync.dma_start(gathp[:, 0:1], zcol[0:NCORES, :])
            for l in (11, 12):
                q[l] = dram.tile([LS[l], T + 1], F32, tag=f"q{l}",
                                 name=f"q{l}")
                nc.sync.dma_start(q[l][:, 0:1], zcol[0:LS[l], :])

            # ---- levels 0..4: standard chunks ----
            def run_groups(pl, lo, hi):
                for i in range(lo, hi, GROUP):
                    grp = pl[i:i + GROUP]
                    for p in range(NPH):
                        for ph in grp:
                            ph[p]()

            prev = None    # (phase_lists, finalize, level, chunk)
            for l in range(N_STD):
                if l not in emit:
                    continue
                rows = min(SZC[l], 128)
                nchunks = max(SZC[l] // 128, 1)
                for c in range(nchunks):
                    dep = False
                    if prev is not None and l > 0 and prev[2] == l - 1:
                        rows_p = min(SZC[l - 1], 128)
                        need = (2 * (c + 1) * rows + rows_p - 1) // rows_p
                        dep = prev[3] < need
                    if prev is not None and dep:
                        run_groups(prev[0], 0, NSLAB)
                        prev[1]()
                        prev = None
                    if prev is not None:
                        run_groups(prev[0], 0, NSLAB - GROUP)
                    consts = _lean_consts(
                        nc, tiny, prm_d[l][c * rows:(c + 1) * rows, :], rows)
                    cur = _std_chunk(nc, pools, consts, lat_d[l],
                                     None if l == 0 else q[l - 1], q[l],
                                     out_padded=(l == N_STD - 1), c=c,
                                     rows=rows)
                    if prev is not None:
                        run_groups(prev[0], NSLAB - GROUP, NSLAB)
                        prev[1]()
                    prev = (cur[0], cur[1], l, c)
            if prev is not None:
                run_groups(prev[0], 0, NSLAB)
                prev[1]()

            # ---- levels 5..13: time-split ----
            for l in range(N_STD, 14):
                if l not in emit:
                    continue
                R, F = SPLIT_ROWS[l], SPLIT_F[l]
                Tseg = T // F
                pstride = T + 1
                prev_dram = gathp if l == 11 else q[l - 1]

                def mk_prev(parity, prev=prev_dram, R=R, F=F, Tseg=Tseg,
                            pstride=pstride):
                    return _ap3(prev[:, :], parity * pstride,
                                [[2 * pstride, R], [Tseg, F], [1, Tseg + 1]])

                if l == 10:
                    def mk_out(qv, ql=q[10], F=F, Tseg=Tseg):
                        dst = _ap3(ql[:, :], 0,
                                   [[T, 1], [Tseg, F], [1, Tseg]])
                        nc.sync.dma_start(dst, qv)
                elif l == 13:
                    def mk_out(qv, F=F, Tseg=Tseg):
                        dst = _ap3(outlet[:, :], 0,
                                   [[T, 1], [Tseg, F], [1, Tseg]])
                        nc.sync.dma_start(dst, qv)
                else:
                    def mk_out(qv, ql=q[l], R=R, F=F, Tseg=Tseg,
                               pstride=pstride):
                        dst = _ap3(ql[:, :], 1,
                                   [[pstride, R], [Tseg, F], [1, Tseg]])
                        nc.sync.dma_start(dst, qv)

                if l == 11:
                    if timeline:
                        for k in range(NCORES):
                            nc.sync.dma_start(gath[k:k + 1, :], q[10][0:1, :])
                    else:
                        nc.gpsimd.collective_compute(
                            "AllGather", ALU.bypass,
                            replica_groups=[list(range(NCORES))],
                            ins=[q[10].opt()], outs=[gath.opt()])
                    nc.sync.dma_start(gathp[:, 1:T + 1], gath[:, :])

                consts = _lean_consts(nc, tiny, prm_d[l][:, :], 128)
                _split_level(nc, pools, psum, consts, ident, lat_d[l],
                             mk_prev, mk_out, l)

    nc.compile()
    return nc


def _make_in_maps(lat, prm_full):
    """lat [T, N] f32, prm_full [N, 7] f32 -> per-core input dicts."""
    in_maps = []
    for k in range(NCORES):
        m = {}
        for l in range(N_STD):
            lo, sz = LO[l], SZC[l]
            sl = slice(lo + k * sz, lo + (k + 1) * sz)
            m[f"lat{l}"] = np.ascontiguousarray(lat[:, sl].T)
            m[f"prm{l}"] = np.ascontiguousarray(prm_full[sl])
        for l in range(N_STD, 14):
            R, F = SPLIT_ROWS[l], SPLIT_F[l]
            Tseg = T // F
            if l < 11:
                lo, sz = LO[l], SZC[l]
                sl = slice(lo + k * sz, lo + (k + 1) * sz)
            else:
                sl = slice(LO[l], LO[l + 1])
            arr = np.ascontiguousarray(lat[:, sl].T)  # [R, T]
            seg = np.zeros((R * F, Tseg + 1), np.float32)
            seg[:, 1:] = arr.reshape(R * F, Tseg)
            s3 = seg.reshape(R, F, Tseg + 1)
            s3[:, 1:, 0] = arr[:, Tseg - 1:T - 1:Tseg]
            m[f"lat{l}"] = seg
            m[f"prm{l}"] = np.ascontiguousarray(
                np.repeat(prm_full[sl], F, axis=0))
        in_maps.append(m)
    return in_maps


_CACHE = {}


def kernel(**inputs):
    lat = np.ascontiguousarray(np.asarray(inputs["lateral_inflows"],
                                          dtype=np.float32))
    prm_full = np.stack([
        np.asarray(inputs["log_manning_n"], np.float32),
        np.asarray(inputs["lengths"], np.float32),
        np.asarray(inputs["slopes"], np.float32),
        np.asarray(inputs["width_coefs"], np.float32),
        np.asarray(inputs["width_exps"], np.float32),
        np.asarray(inputs["depth_coefs"], np.float32),
        np.asarray(inputs["depth_exps"], np.float32),
    ], axis=1)  # [N_REACHES, 7]

    if "nc" not in _CACHE:
        _CACHE["nc"] = _build_program()
    nc = _CACHE["nc"]

    in_maps = _make_in_maps(lat, prm_full)
    res = run_bass_kernel_spmd(nc, in_maps, list(range(NCORES)))
    out = np.asarray(res.results[0]["outlet"]).reshape(T)
    return out.astype(np.float32)


if __name__ == "__main__":
    rng = np.random.default_rng(0)
    fake = dict(
        lateral_inflows=rng.uniform(0, 5, (T, LO[-1])).astype(np.float32),
        log_manning_n=(np.log(0.035) + 0.1 * rng.standard_normal(LO[-1])
                       ).astype(np.float32),
        lengths=rng.uniform(1000, 5000, LO[-1]).astype(np.float32),
        slopes=np.maximum(1e-4, rng.uniform(0.001, 0.003, LO[-1])
                          ).astype(np.float32),
        width_coefs=np.full(LO[-1], 5.0, np.float32),
        width_exps=np.full(LO[-1], 0.5, np.float32),
        depth_coefs=np.full(LO[-1], 0.3, np.float32),
        depth_exps=np.full(LO[-1], 0.4, np.float32),
    )
    out = kernel(**fake)
    print("kernel output head:", out[:4], "tail:", out[-4:])
